# revision 1
# baseline (speedup 1.0000x reference)
"""Trainium2 Bass kernel for nn_ChannelLatentMixer (segment mean + concat).

Reference computation:
    z: (4096, 1, 64, 128) f32, ch_ids: (4096,) int in [0, 32)
    mean[c] = mean of z[b] over rows b with ch_ids[b] == c     (32, 64, 128)
    out = concat([z.squeeze(1), mean[ch_ids]], axis=-2)        (4096, 128, 128)

Strategy: shard the *patch* dimension (64 -> 8 per core) across the 8
NeuronCores.  Each core sees all 4096 batch rows for its 8-patch column
slice, so the segment reduction is fully local — no collective needed.
On each core:
  phase 1: seg-mean as a matmul  mean = onehot_scaled.T @ z_slice
           (onehot_scaled[b,c] = (ch_ids[b]==c)/count[c], host-built),
           accumulated over 32 K-tiles of 128 rows into one PSUM tile.
           Each loaded z tile is also stored straight to the output's
           first half (the concat copy) so z is read from HBM only once.
  phase 2: broadcast-gather as a matmul  aggr = onehot @ mean, done per
           row-tile of 128 rows, PSUM -> SBUF -> DMA to output second half.
Matmuls run in float32r (4-byte fp32 layout, relaxed-precision multiply,
full PE rate at N=512).  All DRAM tensors and SBUF tiles are declared
float32r so the BIR verifier sees every matmul input produced as f32r;
numpy layout is identical to float32.
"""

import numpy as np

import concourse.bacc as bacc
import concourse.mybir as mybir
import concourse.tile as tile
from concourse import bass_utils

F32 = mybir.dt.float32
F32R = mybir.dt.float32r

B = 4096          # batch rows
NPATCH = 64       # patch dim of z
D = 128           # feature dim
C = 32            # num channels
NCORES = 8
PPC = NPATCH // NCORES   # patches per core
COLS = PPC * D           # 1024 columns per core
KT = B // 128            # 32 k-tiles of 128 rows

_compiled = None


def _build_program():
    nc = bacc.Bacc(
        "TRN2", target_bir_lowering=False, debug=False, num_devices=NCORES
    )
    z_d = nc.dram_tensor("z_s", [B, COLS], F32R, kind="ExternalInput").ap()
    oha_d = nc.dram_tensor("oh_a", [128, KT * C], F32R, kind="ExternalInput").ap()
    oht_d = nc.dram_tensor("oh_t", [C, B], F32R, kind="ExternalInput").ap()
    out_d = nc.dram_tensor("out", [B, 2 * COLS], F32R, kind="ExternalOutput").ap()

    z3 = z_d.rearrange("(t p) c -> t p c", p=128)      # [32, 128, 1024]
    out3 = out_d.rearrange("(t p) c -> t p c", p=128)  # [32, 128, 2048]

    with tile.TileContext(nc) as tc:
        with (
            tc.tile_pool(name="cst", bufs=1) as cst,
            tc.tile_pool(name="zp", bufs=16) as zp,
            tc.tile_pool(name="mp", bufs=1) as mp,
            tc.tile_pool(name="ps1", bufs=1, space="PSUM") as ps1,
            tc.tile_pool(name="ps2", bufs=3, space="PSUM") as ps2,
            tc.tile_pool(name="agp", bufs=8) as agp,
        ):
            # constants load on the scalar ring so the first z loads
            # (sync ring) start immediately
            oha = cst.tile([128, KT * C], F32R, tag="oha")
            nc.scalar.dma_start(oha[:], oha_d[:])
            oht = cst.tile([C, B], F32R, tag="oht")
            nc.scalar.dma_start(oht[:], oht_d[:])

            # ---- phase 1: segment sums (pre-scaled -> mean) ----
            acc = ps1.tile([C, COLS], F32)  # 2 PSUM banks
            for k in range(KT):
                zt = zp.tile([128, COLS], F32R, tag="z")
                nc.sync.dma_start(zt[:], z3[k])
                # concat copy: z slice -> first half of output rows.
                # Stores go on the scalar-engine HWDGE ring so loads
                # (sync ring) never wait behind a store's semaphore.
                nc.scalar.dma_start(out3[k, :, 0:COLS], zt[:])
                lw = oha[:, k * C : (k + 1) * C]
                nc.tensor.matmul(
                    acc[:, 0:512], lw, zt[:, 0:512],
                    start=(k == 0), stop=(k == KT - 1),
                )
                nc.tensor.matmul(
                    acc[:, 512:1024], lw, zt[:, 512:1024],
                    start=(k == 0), stop=(k == KT - 1),
                )

            # split the psum->sbuf cast so phase 2's first matmul can
            # start after the first half lands
            mean = mp.tile([C, COLS], F32R, tag="mean")
            nc.vector.tensor_copy(mean[:, 0:512], acc[:, 0:512])
            nc.vector.tensor_copy(mean[:, 512:1024], acc[:, 512:1024])

            # ---- phase 2: broadcast mean back to rows ----
            for t in range(KT):
                pt = ps2.tile([128, COLS], F32, tag="p2")  # 2 PSUM banks
                lw2 = oht[:, t * 128 : (t + 1) * 128]
                nc.tensor.matmul(
                    pt[:, 0:512], lw2, mean[:, 0:512],
                    start=True, stop=True,
                )
                nc.tensor.matmul(
                    pt[:, 512:1024], lw2, mean[:, 512:1024],
                    start=True, stop=True,
                )
                at = agp.tile([128, COLS], F32, tag="a")
                # alternate the PSUM->SBUF evacuation between DVE and ACT:
                # DVE alone (1.22us/tile) paces the stores otherwise
                if t % 2 == 0:
                    nc.vector.tensor_copy(at[:], pt[:])
                else:
                    nc.scalar.copy(at[:], pt[:])
                # alternate rings: two active queues sustain full HBM rate
                eng = nc.scalar if t % 2 == 0 else nc.sync
                eng.dma_start(out3[t, :, COLS : 2 * COLS], at[:].bitcast(F32R))

    nc.compile()
    return nc


def _get_program():
    global _compiled
    if _compiled is None:
        _compiled = _build_program()
    return _compiled


def _host_prep(z, ch_ids):
    z2 = np.ascontiguousarray(np.asarray(z, dtype=np.float32)).reshape(B, NPATCH * D)
    ids = np.asarray(ch_ids).astype(np.int64)
    counts = np.bincount(ids, minlength=C).astype(np.float32)
    scale = 1.0 / np.maximum(counts, 1.0)
    onehot = (ids[:, None] == np.arange(C)[None, :])
    oh_scaled = (onehot * scale[None, :]).astype(np.float32)
    # [128, 32*32]: col block k holds rows k*128..k*128+128 of oh_scaled
    oh_a = np.ascontiguousarray(
        oh_scaled.reshape(KT, 128, C).transpose(1, 0, 2).reshape(128, KT * C)
    )
    # [32, 4096]: lhsT for phase 2 (unscaled onehot, channel-major)
    oh_t = np.ascontiguousarray(onehot.T.astype(np.float32))
    return z2, oh_a, oh_t


def kernel(z, ch_ids):
    z2, oh_a, oh_t = _host_prep(z, ch_ids)
    nc = _get_program()
    in_maps = []
    for m in range(NCORES):
        in_maps.append({
            "z_s": np.ascontiguousarray(z2[:, m * COLS : (m + 1) * COLS]),
            "oh_a": oh_a,
            "oh_t": oh_t,
        })
    res = bass_utils.run_bass_kernel_spmd(
        nc, in_maps, core_ids=list(range(NCORES))
    )
    out = np.empty((B, 2 * NPATCH, D), dtype=np.float32)
    for m in range(NCORES):
        oc = res.results[m]["out"]
        out[:, m * PPC : (m + 1) * PPC, :] = oc[:, :COLS].reshape(B, PPC, D)
        out[:, NPATCH + m * PPC : NPATCH + (m + 1) * PPC, :] = (
            oc[:, COLS:].reshape(B, PPC, D)
        )
    return out



# revision 2
# speedup vs baseline: 1.4700x; 1.4700x over previous
"""Trainium2 Bass kernel for nn_ChannelLatentMixer (segment mean + concat).

Reference computation:
    z: (4096, 1, 64, 128) f32, ch_ids: (4096,) int in [0, 32)
    mean[c] = mean of z[b] over rows b with ch_ids[b] == c     (32, 64, 128)
    out = concat([z.squeeze(1), mean[ch_ids]], axis=-2)        (4096, 128, 128)

Strategy: shard the *patch* dimension (64 -> 8 per core) across the 8
NeuronCores.  Each core sees all 4096 batch rows for its 8-patch column
slice, so the segment reduction is fully local — no collective needed.

The problem is memory-bound (rel-err gate 2e-2), so all device I/O is
bf16: per core 8.4 MB read + 16.8 MB written vs 50.3 MB in f32.
On each core:
  phase 1: seg-mean as a matmul  mean = onehot_scaled.T @ z_slice
           (onehot_scaled[b,c] = (ch_ids[b]==c)/count[c], host-built),
           accumulated over 32 K-tiles of 128 rows into one PSUM tile.
           Each loaded z tile is also stored straight to the output's
           first half (the concat copy) so z is read from HBM only once.
  phase 2: broadcast-gather as a matmul  aggr = onehot @ mean, done per
           row-tile of 128 rows, PSUM -> SBUF (bf16 cast) -> DMA out.
Host converts z to bf16 on the way in and the outputs back to f32.
"""

import numpy as np
import ml_dtypes

import concourse.bacc as bacc
import concourse.mybir as mybir
import concourse.tile as tile
from concourse import bass_utils

F32 = mybir.dt.float32
BF16 = mybir.dt.bfloat16
NP_BF16 = ml_dtypes.bfloat16

B = 4096          # batch rows
NPATCH = 64       # patch dim of z
D = 128           # feature dim
C = 32            # num channels
NCORES = 8
PPC = NPATCH // NCORES   # patches per core
COLS = PPC * D           # 1024 columns per core
KT = B // 128            # 32 k-tiles of 128 rows

_compiled = None


def _build_program():
    nc = bacc.Bacc(
        "TRN2", target_bir_lowering=False, debug=False, num_devices=NCORES
    )
    z_d = nc.dram_tensor("z_s", [B, COLS], BF16, kind="ExternalInput").ap()
    oha_d = nc.dram_tensor("oh_a", [128, KT * C], BF16, kind="ExternalInput").ap()
    oht_d = nc.dram_tensor("oh_t", [C, B], BF16, kind="ExternalInput").ap()
    outz_d = nc.dram_tensor("out_z", [B, COLS], BF16, kind="ExternalOutput").ap()
    outa_d = nc.dram_tensor("out_a", [B, COLS], BF16, kind="ExternalOutput").ap()

    z3 = z_d.rearrange("(t p) c -> t p c", p=128)        # [32, 128, 1024]
    oz3 = outz_d.rearrange("(t p) c -> t p c", p=128)    # [32, 128, 1024]
    oa3 = outa_d.rearrange("(t p) c -> t p c", p=128)    # [32, 128, 1024]

    with tile.TileContext(nc) as tc:
        with (
            tc.tile_pool(name="cst", bufs=1) as cst,
            tc.tile_pool(name="zp", bufs=16) as zp,
            tc.tile_pool(name="mp", bufs=1) as mp,
            tc.tile_pool(name="ps1", bufs=1, space="PSUM") as ps1,
            tc.tile_pool(name="ps2", bufs=3, space="PSUM") as ps2,
            tc.tile_pool(name="agp", bufs=8) as agp,
        ):
            # constants load on the scalar ring so the first z loads
            # (sync ring) start immediately
            oha = cst.tile([128, KT * C], BF16, tag="oha")
            nc.scalar.dma_start(oha[:], oha_d[:])
            oht = cst.tile([C, B], BF16, tag="oht")
            nc.scalar.dma_start(oht[:], oht_d[:])

            # ---- phase 1: segment sums (pre-scaled -> mean) ----
            acc = ps1.tile([C, COLS], F32)  # 2 PSUM banks
            for k in range(KT):
                zt = zp.tile([128, COLS], BF16, tag="z")
                nc.sync.dma_start(zt[:], z3[k])
                # concat copy: z slice -> first half of output.  Stores go
                # on the scalar-engine HWDGE ring so loads (sync ring)
                # never wait behind a store's semaphore.
                nc.scalar.dma_start(oz3[k], zt[:])
                lw = oha[:, k * C : (k + 1) * C]
                nc.tensor.matmul(
                    acc[:, 0:512], lw, zt[:, 0:512],
                    start=(k == 0), stop=(k == KT - 1),
                )
                nc.tensor.matmul(
                    acc[:, 512:1024], lw, zt[:, 512:1024],
                    start=(k == 0), stop=(k == KT - 1),
                )

            # split the psum->sbuf cast so phase 2's first matmul can
            # start after the first half lands
            mean = mp.tile([C, COLS], BF16, tag="mean")
            nc.vector.tensor_copy(mean[:, 0:512], acc[:, 0:512])
            nc.vector.tensor_copy(mean[:, 512:1024], acc[:, 512:1024])

            # ---- phase 2: broadcast mean back to rows ----
            for t in range(KT):
                pt = ps2.tile([128, COLS], F32, tag="p2")  # 2 PSUM banks
                lw2 = oht[:, t * 128 : (t + 1) * 128]
                nc.tensor.matmul(
                    pt[:, 0:512], lw2, mean[:, 0:512],
                    start=True, stop=True,
                )
                nc.tensor.matmul(
                    pt[:, 512:1024], lw2, mean[:, 512:1024],
                    start=True, stop=True,
                )
                at = agp.tile([128, COLS], BF16, tag="a")
                # alternate the PSUM->SBUF evacuation between DVE and ACT
                if t % 2 == 0:
                    nc.vector.tensor_copy(at[:], pt[:])
                else:
                    nc.scalar.copy(at[:], pt[:])
                # alternate rings: two active queues sustain full HBM rate
                eng = nc.scalar if t % 2 == 0 else nc.sync
                eng.dma_start(oa3[t], at[:])

    nc.compile()
    return nc


def _get_program():
    global _compiled
    if _compiled is None:
        _compiled = _build_program()
    return _compiled


def _host_prep(z, ch_ids):
    z2 = np.ascontiguousarray(
        np.asarray(z, dtype=np.float32).reshape(B, NPATCH * D)
    ).astype(NP_BF16)
    ids = np.asarray(ch_ids).astype(np.int64)
    counts = np.bincount(ids, minlength=C).astype(np.float32)
    scale = 1.0 / np.maximum(counts, 1.0)
    onehot = (ids[:, None] == np.arange(C)[None, :])
    oh_scaled = (onehot * scale[None, :]).astype(NP_BF16)
    # [128, 32*32]: col block k holds rows k*128..k*128+128 of oh_scaled
    oh_a = np.ascontiguousarray(
        oh_scaled.reshape(KT, 128, C).transpose(1, 0, 2).reshape(128, KT * C)
    )
    # [32, 4096]: lhsT for phase 2 (unscaled onehot, channel-major)
    oh_t = np.ascontiguousarray(onehot.T.astype(NP_BF16))
    return z2, oh_a, oh_t


def kernel(z, ch_ids):
    z2, oh_a, oh_t = _host_prep(z, ch_ids)
    nc = _get_program()
    in_maps = []
    for m in range(NCORES):
        in_maps.append({
            "z_s": np.ascontiguousarray(z2[:, m * COLS : (m + 1) * COLS]),
            "oh_a": oh_a,
            "oh_t": oh_t,
        })
    res = bass_utils.run_bass_kernel_spmd(
        nc, in_maps, core_ids=list(range(NCORES))
    )
    out = np.empty((B, 2 * NPATCH, D), dtype=np.float32)
    for m in range(NCORES):
        r = res.results[m]
        out[:, m * PPC : (m + 1) * PPC, :] = (
            r["out_z"].astype(np.float32).reshape(B, PPC, D)
        )
        out[:, NPATCH + m * PPC : NPATCH + (m + 1) * PPC, :] = (
            r["out_a"].astype(np.float32).reshape(B, PPC, D)
        )
    return out
